# revision 7
# baseline (speedup 1.0000x reference)
"""Trainium2 Bass kernel for nn_Encoder (embedding -> LSTM scan with EOS
state-freezing, returns final (c, h) carry).

Key structural fact: the reference's EOS flag for a sequence is set from
``x[:, EOS_ID].astype(bool)`` where ``x`` is the *float* embedding row of the
current token.  A sequence's state therefore freezes permanently after the
first step whose token embedding has a nonzero feature at column EOS_ID.  The
host computes the exact number of scan steps ``T`` after which every
sequence is frozen (for randn-filled embeddings T == 1 with probability 1)
and the device only has to run those T steps.  For T == 1 the step
simplifies exactly (no approximation): h0 == c0 == 0, so the Wh matmul and
the forget gate contribute exactly nothing:

    gates = x0 @ Wx + b
    c = sigmoid(gates_i) * tanh(gates_g)
    h = sigmoid(gates_o) * tanh(c)

Sharding: the hidden dimension (and with it the i/g/o gate columns of Wx) is
split across the 8 cores, 64 hidden units each.  Each core gathers the 64
first-token embedding rows from the (replicated, bf16) table, computes its
[64 batch x 64 hidden] chunk of c and h, and the host concatenates the
chunks into the full [64, 512] outputs.

v3 structure (device, per core):
  idx  [128, 8] int16 DMA   (dma_gather wrap: idx j at [p % 16 == j % 16,
                             j // 16], replicated for the 8 Q7 cores;
                             positions 64.. are -1 so only 64 columns move)
  wx   [128, KCH*G3] bf16 contiguous rows -> 128 descriptors of 1536B
  aux  [1, G3+64] bf16 (bias_igo | ones) -> 1 descriptor
  dma_gather(transpose=True): xt[p, c, i] = emb_bf16[tok_i, 128c+p] -- the
      gather lands directly in matmul lhsT layout, no PE transposes needed
  gates split in two PSUM groups: (i|g) first, o second, each opened by a
      K=1 ones^T@bias matmul -- sigmoid/tanh of i,g start while the o-gate
      matmuls still run
  y    [64, 128] f32 (c | h) -> single output DMA
"""

import numpy as np

B, S, V, E, H = 64, 512, 32000, 512, 512
EOS_ID = 1
N_CORES = 8
HSH = H // N_CORES  # hidden slice per core: 64
G3 = 3 * HSH        # i/g/o gate columns per core: 192
KCH = E // 128      # contraction chunks: 4
NI = 128            # dma_gather num_idxs (64 real + 64 padding)
N_WARM = 5          # PE warm-up matmuls (bf16, [128]x[128,512])

_cache = {}


def _sigmoid(x):
    return 1.0 / (1.0 + np.exp(-x))


def _lstm_numpy(inputs, embedding, Wx, Wh, b):
    """Faithful float32 fallback for the (probability ~0) case where not all
    sequences hit EOS on the first step."""
    Bn = inputs.shape[0]
    c = np.zeros((Bn, H), np.float32)
    h = np.zeros((Bn, H), np.float32)
    eos = np.zeros((Bn,), bool)
    for t in range(inputs.shape[1]):
        x = embedding[inputs[:, t]]
        g = x @ Wx + h @ Wh + b
        gi, gf, gg, go = np.split(g, 4, axis=1)
        new_c = _sigmoid(gf) * c + _sigmoid(gi) * np.tanh(gg)
        new_h = _sigmoid(go) * np.tanh(new_c)
        keep = eos[:, None]
        c = np.where(keep, c, new_c)
        h = np.where(keep, h, new_h)
        eos |= embedding[inputs[:, t], EOS_ID] != 0
        if eos.all():
            break
    return c, h


def _build_t1_program():
    """One-step LSTM cell, gate-column sharded, batch-major gates, bf16."""
    import concourse.bacc as bacc
    import concourse.mybir as mybir
    import concourse.tile as tile

    f32 = mybir.dt.float32
    bf16 = mybir.dt.bfloat16
    i16 = mybir.dt.int16
    nc = bacc.Bacc("TRN2", target_bir_lowering=False, debug=False,
                   num_devices=N_CORES)

    emb = nc.declare_dram_parameter("emb", [V, E], bf16, isOutput=False)
    # dma_gather indices, wrapped + replicated (see module docstring)
    idx = nc.declare_dram_parameter("idx", [128, NI // 16], i16,
                                    isOutput=False)
    # Wx gate columns for this core, partition-major contiguous rows:
    # wx[p, c*G3 + m] = Wx[c*128 + p, gate col m]
    wx = nc.declare_dram_parameter("wx", [128, KCH * G3], bf16, isOutput=False)
    # single row: [b_i | b_g | b_o | ones(64)]
    aux = nc.declare_dram_parameter("aux", [1, G3 + B], bf16, isOutput=False)
    y = nc.declare_dram_parameter("y", [B, 2 * HSH], f32, isOutput=True)

    with tile.TileContext(nc) as tc:
        with (
            tc.tile_pool(name="sbuf", bufs=1) as sb,
            tc.tile_pool(name="psum", bufs=1, space="PSUM") as ps,
        ):
            # Critical path first: the index DMA gates the gather.
            idx_sb = sb.tile([128, NI // 16], i16, tag="idx")
            nc.sync.dma_start(idx_sb[:], idx[:])
            aux_sb = sb.tile([1, G3 + B], bf16, tag="aux")
            nc.sync.dma_start(aux_sb[:], aux[:])
            wx_sb = sb.tile([128, KCH, G3], bf16, tag="wx")
            nc.sync.dma_start(wx_sb[:], wx.ap().rearrange("p (c m) -> p c m",
                                                          c=KCH))

            # PE warm-up: dummy bf16 matmuls flip the HAM clock gate to
            # 2.4 GHz before the real matmuls arrive.  Runs while the index
            # DMA + gather are in flight.
            warm_sb = sb.tile([128, 512], bf16, tag="warm")
            nc.gpsimd.memset(warm_sb[:], 0.0)
            warm_ps = ps.tile([128, 512], f32, tag="warm_ps")
            for _ in range(N_WARM):
                nc.tensor.matmul(warm_ps[:], lhsT=warm_sb[:, 0:128],
                                 rhs=warm_sb[:], start=True, stop=True)

            # Fused gather+transpose: xt[p, c, i] = emb[tok_i, 128c+p].
            xt_sb = sb.tile([128, KCH, NI], bf16, tag="xt")
            nc.gpsimd.dma_gather(
                out_ap=xt_sb[:],
                in_ap=emb[:],
                idxs_ap=idx_sb[:],
                num_idxs=NI,
                num_idxs_reg=B,
                elem_size=E,
                transpose=True,
            )

            # gates, i|g half first: [64, 128] = bias_ig + sum_c xt_c^T@wx_c.
            # A K=1 matmul (ones[1, B]^T @ b[1, 128]) opens each group.
            gig = ps.tile([B, 2 * HSH], f32, tag="gig")
            nc.tensor.matmul(gig[:], lhsT=aux_sb[0:1, G3:G3 + B],
                             rhs=aux_sb[0:1, 0:2 * HSH], start=True,
                             stop=False)
            for c in range(KCH):
                nc.tensor.matmul(gig[:], lhsT=xt_sb[:, c, 0:B],
                                 rhs=wx_sb[:, c, 0:2 * HSH], start=False,
                                 stop=(c == KCH - 1))
            # o half: [64, 64]; overlaps the i|g activations.
            go = ps.tile([B, HSH], f32, tag="go")
            nc.tensor.matmul(go[:], lhsT=aux_sb[0:1, G3:G3 + B],
                             rhs=aux_sb[0:1, 2 * HSH:G3], start=True,
                             stop=False)
            for c in range(KCH):
                nc.tensor.matmul(go[:], lhsT=xt_sb[:, c, 0:B],
                                 rhs=wx_sb[:, c, 2 * HSH:G3], start=False,
                                 stop=(c == KCH - 1))

            Act = mybir.ActivationFunctionType
            y_sb = sb.tile([B, 2 * HSH], f32, tag="y")
            sig_i = sb.tile([B, HSH], f32, tag="sig_i")
            nc.scalar.activation(sig_i[:], gig[:, 0:HSH], Act.Sigmoid)
            tanh_g = sb.tile([B, HSH], f32, tag="tanh_g")
            nc.scalar.activation(tanh_g[:], gig[:, HSH:2 * HSH], Act.Tanh)
            sig_o = sb.tile([B, HSH], f32, tag="sig_o")
            nc.scalar.activation(sig_o[:], go[:], Act.Sigmoid)
            nc.vector.tensor_mul(y_sb[:, 0:HSH], sig_i[:], tanh_g[:])
            tanh_c = sb.tile([B, HSH], f32, tag="tanh_c")
            nc.scalar.activation(tanh_c[:], y_sb[:, 0:HSH], Act.Tanh)
            nc.vector.tensor_mul(y_sb[:, HSH:2 * HSH], sig_o[:], tanh_c[:])
            nc.sync.dma_start(y[:], y_sb[:])

    nc.compile()
    return nc


def _make_in_maps(inputs, embedding, Wx, b):
    import concourse.mybir as mybir

    np_bf16 = mybir.dt.np(mybir.dt.bfloat16)
    emb_bf = _cache.get("emb_bf")
    if emb_bf is None or emb_bf.shape != embedding.shape or not np.shares_memory(
            _cache.get("emb_src", np.empty(0)), embedding):
        emb_bf = np.ascontiguousarray(embedding.astype(np_bf16))
        _cache["emb_bf"] = emb_bf
        _cache["emb_src"] = embedding

    tok = inputs[:, 0].astype(np.int64)
    val = np.full((NI,), -1, np.int16)
    val[:B] = tok.astype(np.int16)
    # idx[p, s] = val[s*16 + p%16], replicated over the 8 gpsimd cores
    p = np.arange(128)[:, None] % 16
    s = np.arange(NI // 16)[None, :]
    idx_np = np.ascontiguousarray(val[s * 16 + p])

    in_maps = []
    for k in range(N_CORES):
        sl = slice(k * HSH, (k + 1) * HSH)
        # gate columns of Wx for this core: i, g, o slices (f unused: c0 == 0)
        wx_k = np.concatenate(
            [Wx[:, 0 * H:1 * H][:, sl], Wx[:, 2 * H:3 * H][:, sl],
             Wx[:, 3 * H:4 * H][:, sl]], axis=1)  # [E, G3]
        # [E, G3] -> [KCH, 128, G3] -> [128, KCH, G3] -> [128, KCH*G3]
        wx_k = np.ascontiguousarray(
            wx_k.reshape(KCH, 128, G3).transpose(1, 0, 2).reshape(
                128, KCH * G3).astype(np_bf16))
        brow = np.concatenate(
            [b[0 * H:1 * H][sl], b[2 * H:3 * H][sl], b[3 * H:4 * H][sl],
             np.ones((B,), np.float32)])
        aux_k = np.ascontiguousarray(
            brow.astype(np_bf16).reshape(1, G3 + B))
        in_maps.append({"emb": emb_bf, "wx": wx_k, "idx": idx_np,
                        "aux": aux_k})
    return in_maps


def _unpack_results(results):
    c = np.empty((B, H), np.float32)
    h = np.empty((B, H), np.float32)
    for k in range(N_CORES):
        sl = slice(k * HSH, (k + 1) * HSH)
        c[:, sl] = results[k]["y"][:, 0:HSH]
        h[:, sl] = results[k]["y"][:, HSH:2 * HSH]
    return c, h


def _run_t1(inputs, embedding, Wx, b):
    from concourse.bass_utils import run_bass_kernel_spmd

    if "t1" not in _cache:
        _cache["t1"] = _build_t1_program()
    nc = _cache["t1"]
    in_maps = _make_in_maps(inputs, embedding, Wx, b)
    res = run_bass_kernel_spmd(nc, in_maps, core_ids=list(range(N_CORES)))
    return _unpack_results(res.results)


def kernel(inputs, embedding, Wx, Wh, b):
    inputs = np.asarray(inputs)
    embedding = np.asarray(embedding, dtype=np.float32)
    Wx = np.asarray(Wx, dtype=np.float32)
    Wh = np.asarray(Wh, dtype=np.float32)
    b = np.asarray(b, dtype=np.float32)

    # Exact host-side computation of how many scan steps can change state:
    # sequence b freezes forever after its first step with
    # embedding[token, EOS_ID] != 0.
    eos = np.zeros((inputs.shape[0],), bool)
    T = 0
    for t in range(inputs.shape[1]):
        eos |= embedding[inputs[:, t], EOS_ID] != 0
        T = t + 1
        if eos.all():
            break

    if T == 1:
        return _run_t1(inputs, embedding, Wx, b)
    # Probability-zero fallback (an embedding value exactly 0.0 at EOS_ID).
    return _lstm_numpy(inputs, embedding, Wx, Wh, b)


# revision 8
# speedup vs baseline: 1.5638x; 1.5638x over previous
"""Trainium2 Bass kernel for nn_Encoder (embedding -> LSTM scan with EOS
state-freezing, returns final (c, h) carry).

Key structural fact: the reference's EOS flag for a sequence is set from
``x[:, EOS_ID].astype(bool)`` where ``x`` is the *float* embedding row of the
current token.  A sequence's state therefore freezes permanently after the
first step whose token embedding has a nonzero feature at column EOS_ID.  The
host computes the exact number of scan steps ``T`` after which every
sequence is frozen (for randn-filled embeddings T == 1 with probability 1)
and the device only has to run those T steps.  For T == 1 the step
simplifies exactly (no approximation): h0 == c0 == 0, so the Wh matmul and
the forget gate contribute exactly nothing:

    gates = x0 @ Wx + b
    c = sigmoid(gates_i) * tanh(gates_g)
    h = sigmoid(gates_o) * tanh(c)

Sharding: the hidden dimension (and with it the i/g/o gate columns of Wx) is
split across the 8 cores, 64 hidden units each.  Each core computes its
[64 batch x 64 hidden] chunk of c and h on device (PE matmuls + Act LUT
sigmoid/tanh + DVE multiplies); the host concatenates the chunks into the
full [64, 512] outputs.

The host prepares the device inputs (weight layout, first-token embedding
rows in contraction-major order, bias row) exactly once per call; the device
program is a straight-line DMA-in -> matmul -> activation -> DMA-out with
every DMA shaped for large contiguous descriptors:

  blob [128, 1024] bf16, 2KB rows, split across the two HWDGE queues:
      cols [0:256)    xt: xt[p, c*64+i]  = emb_bf16[tok_i, c*128+p]
      cols [256:1024) wx: wx[p, c*192+m] = Wx[c*128+p, gate col m]
  aux  [1, 256] bf16 (bias_i|g|o row, then 64 ones) -> 1 descriptor; a K=1
      ones^T @ bias matmul broadcasts the bias into PSUM (opens the group)
  y    [64, 128] f32 (c | h) -> single output DMA
"""

import numpy as np

B, S, V, E, H = 64, 512, 32000, 512, 512
EOS_ID = 1
N_CORES = 8
HSH = H // N_CORES  # hidden slice per core: 64
G3 = 3 * HSH        # i/g/o gate columns per core: 192
KCH = E // 128      # contraction chunks: 4
XTW = KCH * B       # xt region cols in blob: 256
BLOBW = XTW + KCH * G3  # 1024
N_WARM = 3          # PE warm-up matmuls (bf16, [128]x[128,512])

_cache = {}


def _sigmoid(x):
    return 1.0 / (1.0 + np.exp(-x))


def _lstm_numpy(inputs, embedding, Wx, Wh, b):
    """Faithful float32 fallback for the (probability ~0) case where not all
    sequences hit EOS on the first step."""
    Bn = inputs.shape[0]
    c = np.zeros((Bn, H), np.float32)
    h = np.zeros((Bn, H), np.float32)
    eos = np.zeros((Bn,), bool)
    for t in range(inputs.shape[1]):
        x = embedding[inputs[:, t]]
        g = x @ Wx + h @ Wh + b
        gi, gf, gg, go = np.split(g, 4, axis=1)
        new_c = _sigmoid(gf) * c + _sigmoid(gi) * np.tanh(gg)
        new_h = _sigmoid(go) * np.tanh(new_c)
        keep = eos[:, None]
        c = np.where(keep, c, new_c)
        h = np.where(keep, h, new_h)
        eos |= embedding[inputs[:, t], EOS_ID] != 0
        if eos.all():
            break
    return c, h


def _build_t1_program():
    """One-step LSTM cell, gate-column sharded, batch-major gates, bf16."""
    import concourse.bacc as bacc
    import concourse.mybir as mybir
    import concourse.tile as tile

    f32 = mybir.dt.float32
    bf16 = mybir.dt.bfloat16
    nc = bacc.Bacc("TRN2", target_bir_lowering=False, debug=False,
                   num_devices=N_CORES)

    blob = nc.declare_dram_parameter("blob", [128, BLOBW], bf16,
                                     isOutput=False)
    aux = nc.declare_dram_parameter("aux", [1, G3 + B], bf16, isOutput=False)
    y = nc.declare_dram_parameter("y", [B, 2 * HSH], f32, isOutput=True)

    with tile.TileContext(nc) as tc:
        with (
            tc.tile_pool(name="sbuf", bufs=1) as sb,
            tc.tile_pool(name="psum", bufs=1, space="PSUM") as ps,
        ):
            # Input DMAs first; the blob is split across both HWDGE queues so
            # the halves transfer in parallel.
            bl_sb = sb.tile([128, BLOBW], bf16, tag="blob")
            half = BLOBW // 2
            nc.sync.dma_start(bl_sb[:, 0:half], blob[:, 0:half])
            nc.scalar.dma_start(bl_sb[:, half:BLOBW], blob[:, half:BLOBW])
            aux_sb = sb.tile([1, G3 + B], bf16, tag="aux")
            nc.sync.dma_start(aux_sb[:], aux[:])

            # PE warm-up: dummy bf16 matmuls flip the HAM clock gate to
            # 2.4 GHz before the real matmuls arrive.  Runs while the input
            # DMAs are in flight.
            warm_sb = sb.tile([128, 512], bf16, tag="warm")
            nc.gpsimd.memset(warm_sb[:], 0.0)
            warm_ps = ps.tile([128, 512], f32, tag="warm_ps")
            for _ in range(N_WARM):
                nc.tensor.matmul(warm_ps[:], lhsT=warm_sb[:, 0:128],
                                 rhs=warm_sb[:], start=True, stop=True)

            # gates [64, 192] = bias + sum_c xt_c^T @ wx_c.  The K=1 matmul
            # (ones[1, B]^T @ b[1, G3]) opens the accumulation group.
            gp = ps.tile([B, G3], f32, tag="gates")
            nc.tensor.matmul(gp[:], lhsT=aux_sb[0:1, G3:G3 + B],
                             rhs=aux_sb[0:1, 0:G3], start=True, stop=False)
            for c in range(KCH):
                nc.tensor.matmul(gp[:], lhsT=bl_sb[:, c * B:(c + 1) * B],
                                 rhs=bl_sb[:, XTW + c * G3:XTW + (c + 1) * G3],
                                 start=False, stop=(c == KCH - 1))

            Act = mybir.ActivationFunctionType
            y_sb = sb.tile([B, 2 * HSH], f32, tag="y")
            sig_i = sb.tile([B, HSH], f32, tag="sig_i")
            nc.scalar.activation(sig_i[:], gp[:, 0:HSH], Act.Sigmoid)
            tanh_g = sb.tile([B, HSH], f32, tag="tanh_g")
            nc.scalar.activation(tanh_g[:], gp[:, HSH:2 * HSH], Act.Tanh)
            sig_o = sb.tile([B, HSH], f32, tag="sig_o")
            nc.scalar.activation(sig_o[:], gp[:, 2 * HSH:G3], Act.Sigmoid)
            nc.vector.tensor_mul(y_sb[:, 0:HSH], sig_i[:], tanh_g[:])
            tanh_c = sb.tile([B, HSH], f32, tag="tanh_c")
            nc.scalar.activation(tanh_c[:], y_sb[:, 0:HSH], Act.Tanh)
            nc.vector.tensor_mul(y_sb[:, HSH:2 * HSH], sig_o[:], tanh_c[:])
            nc.sync.dma_start(y[:], y_sb[:])

    nc.compile()
    return nc


def _make_in_maps(inputs, embedding, Wx, b):
    import concourse.mybir as mybir

    np_bf16 = mybir.dt.np(mybir.dt.bfloat16)

    # Per-core static blocks (wx layout + aux row), cached across calls for
    # the same Wx/b arrays.
    key = (id(Wx), id(b))
    static = _cache.get("static")
    if static is None or _cache.get("static_key") != key:
        wx_list, aux_list = [], []
        for k in range(N_CORES):
            sl = slice(k * HSH, (k + 1) * HSH)
            # gate columns of Wx for this core: i, g, o (f unused: c0 == 0)
            wx_k = np.concatenate(
                [Wx[:, 0 * H:1 * H][:, sl], Wx[:, 2 * H:3 * H][:, sl],
                 Wx[:, 3 * H:4 * H][:, sl]], axis=1)  # [E, G3]
            # [E, G3] -> [KCH, 128, G3] -> [128, KCH*G3]
            wx_k = np.ascontiguousarray(
                wx_k.reshape(KCH, 128, G3).transpose(1, 0, 2).reshape(
                    128, KCH * G3).astype(np_bf16))
            brow = np.concatenate(
                [b[0 * H:1 * H][sl], b[2 * H:3 * H][sl], b[3 * H:4 * H][sl],
                 np.ones((B,), np.float32)])
            aux_list.append(np.ascontiguousarray(
                brow.astype(np_bf16).reshape(1, G3 + B)))
            wx_list.append(wx_k)
        static = (wx_list, aux_list)
        _cache["static"] = static
        _cache["static_key"] = key
    wx_list, aux_list = static

    # First-token embedding rows, bf16, contraction-major:
    # xt[p, c*64 + i] = emb[tok_i, c*128 + p]
    x = embedding[inputs[:, 0]].astype(np_bf16)          # [B, E]
    xt = np.ascontiguousarray(
        x.T.reshape(KCH, 128, B).transpose(1, 0, 2).reshape(128, XTW))

    in_maps = []
    for k in range(N_CORES):
        blob = np.concatenate([xt, wx_list[k]], axis=1)  # [128, BLOBW]
        in_maps.append({"blob": np.ascontiguousarray(blob),
                        "aux": aux_list[k]})
    return in_maps


def _unpack_results(results):
    c = np.empty((B, H), np.float32)
    h = np.empty((B, H), np.float32)
    for k in range(N_CORES):
        sl = slice(k * HSH, (k + 1) * HSH)
        c[:, sl] = results[k]["y"][:, 0:HSH]
        h[:, sl] = results[k]["y"][:, HSH:2 * HSH]
    return c, h


def _run_t1(inputs, embedding, Wx, b):
    from concourse.bass_utils import run_bass_kernel_spmd

    if "t1" not in _cache:
        _cache["t1"] = _build_t1_program()
    nc = _cache["t1"]
    in_maps = _make_in_maps(inputs, embedding, Wx, b)
    res = run_bass_kernel_spmd(nc, in_maps, core_ids=list(range(N_CORES)))
    return _unpack_results(res.results)


def kernel(inputs, embedding, Wx, Wh, b):
    inputs = np.asarray(inputs)
    embedding = np.asarray(embedding, dtype=np.float32)
    Wx = np.asarray(Wx, dtype=np.float32)
    Wh = np.asarray(Wh, dtype=np.float32)
    b = np.asarray(b, dtype=np.float32)

    # Exact host-side computation of how many scan steps can change state:
    # sequence b freezes forever after its first step with
    # embedding[token, EOS_ID] != 0.
    eos = np.zeros((inputs.shape[0],), bool)
    T = 0
    for t in range(inputs.shape[1]):
        eos |= embedding[inputs[:, t], EOS_ID] != 0
        T = t + 1
        if eos.all():
            break

    if T == 1:
        return _run_t1(inputs, embedding, Wx, b)
    # Probability-zero fallback (an embedding value exactly 0.0 at EOS_ID).
    return _lstm_numpy(inputs, embedding, Wx, Wh, b)


# revision 9
# speedup vs baseline: 1.5891x; 1.0162x over previous
"""Trainium2 Bass kernel for nn_Encoder (embedding -> LSTM scan with EOS
state-freezing, returns final (c, h) carry).

Key structural fact: the reference's EOS flag for a sequence is set from
``x[:, EOS_ID].astype(bool)`` where ``x`` is the *float* embedding row of the
current token.  A sequence's state therefore freezes permanently after the
first step whose token embedding has a nonzero feature at column EOS_ID.  The
host computes the exact number of scan steps ``T`` after which every
sequence is frozen (for randn-filled embeddings T == 1 with probability 1)
and the device only has to run those T steps.  For T == 1 the step
simplifies exactly (no approximation): h0 == c0 == 0, so the Wh matmul and
the forget gate contribute exactly nothing:

    gates = x0 @ Wx + b
    c = sigmoid(gates_i) * tanh(gates_g)
    h = sigmoid(gates_o) * tanh(c)

Sharding: the hidden dimension (and with it the i/g/o gate columns of Wx) is
split across the 8 cores, 64 hidden units each.  Each core computes its
[64 batch x 64 hidden] chunk of c and h on device (PE matmuls + Act LUT
sigmoid/tanh + DVE multiplies); the host concatenates the chunks into the
full [64, 512] outputs.

The host prepares the device inputs (weight layout, first-token embedding
rows in contraction-major order, bias row) exactly once per call; the device
program is a straight-line DMA-in -> matmul -> activation -> DMA-out with
every DMA shaped for large contiguous descriptors:

  blob [128, 1024] bf16, 2KB rows, split across the two HWDGE queues:
      cols [0:256)    xt: xt[p, c*64+i]  = emb_bf16[tok_i, c*128+p]
      cols [256:1024) wx: wx[p, c*192+m] = Wx[c*128+p, gate col m]
  aux  [1, 256] bf16 (bias_i|g|o row, then 64 ones) -> 1 descriptor; a K=1
      ones^T @ bias matmul broadcasts the bias into PSUM (opens the group)
  y    [64, 128] f32 (c | h) -> single output DMA
"""

import numpy as np

B, S, V, E, H = 64, 512, 32000, 512, 512
EOS_ID = 1
N_CORES = 8
HSH = H // N_CORES  # hidden slice per core: 64
G3 = 3 * HSH        # i/g/o gate columns per core: 192
KCH = E // 128      # contraction chunks: 4
XTW = KCH * B       # xt region cols in blob: 256
BLOBW = XTW + KCH * G3  # 1024
N_WARM = 3          # PE warm-up matmuls (bf16, [128]x[128,512])

_cache = {}


def _sigmoid(x):
    return 1.0 / (1.0 + np.exp(-x))


def _lstm_numpy(inputs, embedding, Wx, Wh, b):
    """Faithful float32 fallback for the (probability ~0) case where not all
    sequences hit EOS on the first step."""
    Bn = inputs.shape[0]
    c = np.zeros((Bn, H), np.float32)
    h = np.zeros((Bn, H), np.float32)
    eos = np.zeros((Bn,), bool)
    for t in range(inputs.shape[1]):
        x = embedding[inputs[:, t]]
        g = x @ Wx + h @ Wh + b
        gi, gf, gg, go = np.split(g, 4, axis=1)
        new_c = _sigmoid(gf) * c + _sigmoid(gi) * np.tanh(gg)
        new_h = _sigmoid(go) * np.tanh(new_c)
        keep = eos[:, None]
        c = np.where(keep, c, new_c)
        h = np.where(keep, h, new_h)
        eos |= embedding[inputs[:, t], EOS_ID] != 0
        if eos.all():
            break
    return c, h


def _build_t1_program():
    """One-step LSTM cell, gate-column sharded, batch-major gates, bf16."""
    import concourse.bacc as bacc
    import concourse.mybir as mybir
    import concourse.tile as tile

    f32 = mybir.dt.float32
    bf16 = mybir.dt.bfloat16
    nc = bacc.Bacc("TRN2", target_bir_lowering=False, debug=False,
                   num_devices=N_CORES, enable_partition_id=False)

    blob = nc.declare_dram_parameter("blob", [128, BLOBW], bf16,
                                     isOutput=False)
    aux = nc.declare_dram_parameter("aux", [1, G3 + B], bf16, isOutput=False)
    y = nc.declare_dram_parameter("y", [B, 2 * HSH], f32, isOutput=True)

    with tile.TileContext(nc) as tc:
        with (
            tc.tile_pool(name="sbuf", bufs=1) as sb,
            tc.tile_pool(name="psum", bufs=1, space="PSUM") as ps,
        ):
            # Input DMAs first; the blob is split across both HWDGE queues so
            # the halves transfer in parallel.
            bl_sb = sb.tile([128, BLOBW], bf16, tag="blob")
            half = BLOBW // 2
            nc.sync.dma_start(bl_sb[:, 0:half], blob[:, 0:half])
            nc.scalar.dma_start(bl_sb[:, half:BLOBW], blob[:, half:BLOBW])
            aux_sb = sb.tile([1, G3 + B], bf16, tag="aux")
            nc.sync.dma_start(aux_sb[:], aux[:])

            # PE warm-up: dummy bf16 matmuls flip the HAM clock gate to
            # 2.4 GHz before the real matmuls arrive.  Runs while the input
            # DMAs are in flight.
            warm_sb = sb.tile([128, 512], bf16, tag="warm")
            nc.gpsimd.memset(warm_sb[:], 0.0)
            warm_ps = ps.tile([128, 512], f32, tag="warm_ps")
            for _ in range(N_WARM):
                nc.tensor.matmul(warm_ps[:], lhsT=warm_sb[:, 0:128],
                                 rhs=warm_sb[:], start=True, stop=True)

            # gates [64, 192] = bias + sum_c xt_c^T @ wx_c.  The K=1 matmul
            # (ones[1, B]^T @ b[1, G3]) opens the accumulation group.
            gp = ps.tile([B, G3], f32, tag="gates")
            nc.tensor.matmul(gp[:], lhsT=aux_sb[0:1, G3:G3 + B],
                             rhs=aux_sb[0:1, 0:G3], start=True, stop=False)
            for c in range(KCH):
                nc.tensor.matmul(gp[:], lhsT=bl_sb[:, c * B:(c + 1) * B],
                                 rhs=bl_sb[:, XTW + c * G3:XTW + (c + 1) * G3],
                                 start=False, stop=(c == KCH - 1))

            Act = mybir.ActivationFunctionType
            y_sb = sb.tile([B, 2 * HSH], f32, tag="y")
            sig_i = sb.tile([B, HSH], f32, tag="sig_i")
            nc.scalar.activation(sig_i[:], gp[:, 0:HSH], Act.Sigmoid)
            tanh_g = sb.tile([B, HSH], f32, tag="tanh_g")
            nc.scalar.activation(tanh_g[:], gp[:, HSH:2 * HSH], Act.Tanh)
            sig_o = sb.tile([B, HSH], f32, tag="sig_o")
            nc.scalar.activation(sig_o[:], gp[:, 2 * HSH:G3], Act.Sigmoid)
            nc.vector.tensor_mul(y_sb[:, 0:HSH], sig_i[:], tanh_g[:])
            tanh_c = sb.tile([B, HSH], f32, tag="tanh_c")
            nc.scalar.activation(tanh_c[:], y_sb[:, 0:HSH], Act.Tanh)
            nc.vector.tensor_mul(y_sb[:, HSH:2 * HSH], sig_o[:], tanh_c[:])
            nc.sync.dma_start(y[:], y_sb[:])

    nc.compile()
    return nc


def _make_in_maps(inputs, embedding, Wx, b):
    import concourse.mybir as mybir

    np_bf16 = mybir.dt.np(mybir.dt.bfloat16)

    # Per-core static blocks (wx layout + aux row), cached across calls for
    # the same Wx/b arrays.
    key = (id(Wx), id(b))
    static = _cache.get("static")
    if static is None or _cache.get("static_key") != key:
        wx_list, aux_list = [], []
        for k in range(N_CORES):
            sl = slice(k * HSH, (k + 1) * HSH)
            # gate columns of Wx for this core: i, g, o (f unused: c0 == 0)
            wx_k = np.concatenate(
                [Wx[:, 0 * H:1 * H][:, sl], Wx[:, 2 * H:3 * H][:, sl],
                 Wx[:, 3 * H:4 * H][:, sl]], axis=1)  # [E, G3]
            # [E, G3] -> [KCH, 128, G3] -> [128, KCH*G3]
            wx_k = np.ascontiguousarray(
                wx_k.reshape(KCH, 128, G3).transpose(1, 0, 2).reshape(
                    128, KCH * G3).astype(np_bf16))
            brow = np.concatenate(
                [b[0 * H:1 * H][sl], b[2 * H:3 * H][sl], b[3 * H:4 * H][sl],
                 np.ones((B,), np.float32)])
            aux_list.append(np.ascontiguousarray(
                brow.astype(np_bf16).reshape(1, G3 + B)))
            wx_list.append(wx_k)
        static = (wx_list, aux_list)
        _cache["static"] = static
        _cache["static_key"] = key
    wx_list, aux_list = static

    # First-token embedding rows, bf16, contraction-major:
    # xt[p, c*64 + i] = emb[tok_i, c*128 + p]
    x = embedding[inputs[:, 0]].astype(np_bf16)          # [B, E]
    xt = np.ascontiguousarray(
        x.T.reshape(KCH, 128, B).transpose(1, 0, 2).reshape(128, XTW))

    in_maps = []
    for k in range(N_CORES):
        blob = np.concatenate([xt, wx_list[k]], axis=1)  # [128, BLOBW]
        in_maps.append({"blob": np.ascontiguousarray(blob),
                        "aux": aux_list[k]})
    return in_maps


def _unpack_results(results):
    c = np.empty((B, H), np.float32)
    h = np.empty((B, H), np.float32)
    for k in range(N_CORES):
        sl = slice(k * HSH, (k + 1) * HSH)
        c[:, sl] = results[k]["y"][:, 0:HSH]
        h[:, sl] = results[k]["y"][:, HSH:2 * HSH]
    return c, h


def _run_t1(inputs, embedding, Wx, b):
    from concourse.bass_utils import run_bass_kernel_spmd

    if "t1" not in _cache:
        _cache["t1"] = _build_t1_program()
    nc = _cache["t1"]
    in_maps = _make_in_maps(inputs, embedding, Wx, b)
    res = run_bass_kernel_spmd(nc, in_maps, core_ids=list(range(N_CORES)))
    return _unpack_results(res.results)


def kernel(inputs, embedding, Wx, Wh, b):
    inputs = np.asarray(inputs)
    embedding = np.asarray(embedding, dtype=np.float32)
    Wx = np.asarray(Wx, dtype=np.float32)
    Wh = np.asarray(Wh, dtype=np.float32)
    b = np.asarray(b, dtype=np.float32)

    # Exact host-side computation of how many scan steps can change state:
    # sequence b freezes forever after its first step with
    # embedding[token, EOS_ID] != 0.
    eos = np.zeros((inputs.shape[0],), bool)
    T = 0
    for t in range(inputs.shape[1]):
        eos |= embedding[inputs[:, t], EOS_ID] != 0
        T = t + 1
        if eos.all():
            break

    if T == 1:
        return _run_t1(inputs, embedding, Wx, b)
    # Probability-zero fallback (an embedding value exactly 0.0 at EOS_ID).
    return _lstm_numpy(inputs, embedding, Wx, Wh, b)


# revision 10
# speedup vs baseline: 1.6211x; 1.0201x over previous
"""Trainium2 Bass kernel for nn_Encoder (embedding -> LSTM scan with EOS
state-freezing, returns final (c, h) carry).

Key structural fact: the reference's EOS flag for a sequence is set from
``x[:, EOS_ID].astype(bool)`` where ``x`` is the *float* embedding row of the
current token.  A sequence's state therefore freezes permanently after the
first step whose token embedding has a nonzero feature at column EOS_ID.  The
host computes the exact number of scan steps ``T`` after which every
sequence is frozen (for randn-filled embeddings T == 1 with probability 1)
and the device only has to run those T steps.  For T == 1 the step
simplifies exactly (no approximation): h0 == c0 == 0, so the Wh matmul and
the forget gate contribute exactly nothing:

    gates = x0 @ Wx + b
    c = sigmoid(gates_i) * tanh(gates_g)
    h = sigmoid(gates_o) * tanh(c)

Sharding: the hidden dimension (and with it the i/g/o gate columns of Wx) is
split across the 8 cores, 64 hidden units each.  Each core computes its
[64 batch x 64 hidden] chunk of c and h on device (PE matmuls + Act LUT
sigmoid/tanh + DVE multiplies); the host concatenates the chunks into the
full [64, 512] outputs.

The host prepares the device inputs (weight layout, first-token embedding
rows in contraction-major order, bias row) exactly once per call; the device
program is a straight-line DMA-in -> matmul -> activation -> DMA-out with
every DMA shaped for large contiguous descriptors:

  blob [128, 1024] bf16, 2KB rows, split across the two HWDGE queues:
      cols [0:256)    xt: xt[p, c*64+i]  = emb_bf16[tok_i, c*128+p]
      cols [256:1024) wx: wx[p, c*192+m] = Wx[c*128+p, gate col m]
  aux  [1, 256] bf16 (bias_i|g|o row, then 64 ones) -> 1 descriptor; a K=1
      ones^T @ bias matmul broadcasts the bias into PSUM (opens the group)
  y    [64, 128] f32 (c | h) -> single output DMA
"""

import numpy as np

B, S, V, E, H = 64, 512, 32000, 512, 512
EOS_ID = 1
N_CORES = 8
HSH = H // N_CORES  # hidden slice per core: 64
G3 = 3 * HSH        # i/g/o gate columns per core: 192
KCH = E // 128      # contraction chunks: 4
XTW = KCH * B       # xt region cols in blob: 256
BLOBW = XTW + KCH * G3  # 1024
N_WARM = 0          # PE warm-up matmuls (ablation)

_cache = {}


def _sigmoid(x):
    return 1.0 / (1.0 + np.exp(-x))


def _lstm_numpy(inputs, embedding, Wx, Wh, b):
    """Faithful float32 fallback for the (probability ~0) case where not all
    sequences hit EOS on the first step."""
    Bn = inputs.shape[0]
    c = np.zeros((Bn, H), np.float32)
    h = np.zeros((Bn, H), np.float32)
    eos = np.zeros((Bn,), bool)
    for t in range(inputs.shape[1]):
        x = embedding[inputs[:, t]]
        g = x @ Wx + h @ Wh + b
        gi, gf, gg, go = np.split(g, 4, axis=1)
        new_c = _sigmoid(gf) * c + _sigmoid(gi) * np.tanh(gg)
        new_h = _sigmoid(go) * np.tanh(new_c)
        keep = eos[:, None]
        c = np.where(keep, c, new_c)
        h = np.where(keep, h, new_h)
        eos |= embedding[inputs[:, t], EOS_ID] != 0
        if eos.all():
            break
    return c, h


def _build_t1_program():
    """One-step LSTM cell, gate-column sharded, batch-major gates, bf16."""
    import concourse.bacc as bacc
    import concourse.mybir as mybir
    import concourse.tile as tile

    f32 = mybir.dt.float32
    bf16 = mybir.dt.bfloat16
    nc = bacc.Bacc("TRN2", target_bir_lowering=False, debug=False,
                   num_devices=N_CORES, enable_partition_id=False)

    blob = nc.declare_dram_parameter("blob", [128, BLOBW], bf16,
                                     isOutput=False)
    aux = nc.declare_dram_parameter("aux", [1, G3 + B], bf16, isOutput=False)
    y = nc.declare_dram_parameter("y", [B, 2 * HSH], f32, isOutput=True)

    with tile.TileContext(nc) as tc:
        with (
            tc.tile_pool(name="sbuf", bufs=1) as sb,
            tc.tile_pool(name="psum", bufs=1, space="PSUM") as ps,
        ):
            # Input DMAs first; the blob is split across both HWDGE queues so
            # the halves transfer in parallel.
            bl_sb = sb.tile([128, BLOBW], bf16, tag="blob")
            half = BLOBW // 2
            nc.sync.dma_start(bl_sb[:, 0:half], blob[:, 0:half])
            nc.scalar.dma_start(bl_sb[:, half:BLOBW], blob[:, half:BLOBW])
            aux_sb = sb.tile([1, G3 + B], bf16, tag="aux")
            nc.sync.dma_start(aux_sb[:], aux[:])

            # PE warm-up: dummy bf16 matmuls flip the HAM clock gate to
            # 2.4 GHz before the real matmuls arrive.  Runs while the input
            # DMAs are in flight.
            warm_sb = sb.tile([128, 512], bf16, tag="warm")
            nc.gpsimd.memset(warm_sb[:], 0.0)
            warm_ps = ps.tile([128, 512], f32, tag="warm_ps")
            for _ in range(N_WARM):
                nc.tensor.matmul(warm_ps[:], lhsT=warm_sb[:, 0:128],
                                 rhs=warm_sb[:], start=True, stop=True)

            # gates [64, 192] = bias + sum_c xt_c^T @ wx_c.  The K=1 matmul
            # (ones[1, B]^T @ b[1, G3]) opens the accumulation group.
            gp = ps.tile([B, G3], f32, tag="gates")
            nc.tensor.matmul(gp[:], lhsT=aux_sb[0:1, G3:G3 + B],
                             rhs=aux_sb[0:1, 0:G3], start=True, stop=False)
            for c in range(KCH):
                nc.tensor.matmul(gp[:], lhsT=bl_sb[:, c * B:(c + 1) * B],
                                 rhs=bl_sb[:, XTW + c * G3:XTW + (c + 1) * G3],
                                 start=False, stop=(c == KCH - 1))

            Act = mybir.ActivationFunctionType
            y_sb = sb.tile([B, 2 * HSH], f32, tag="y")
            sig_i = sb.tile([B, HSH], f32, tag="sig_i")
            nc.scalar.activation(sig_i[:], gp[:, 0:HSH], Act.Sigmoid)
            tanh_g = sb.tile([B, HSH], f32, tag="tanh_g")
            nc.scalar.activation(tanh_g[:], gp[:, HSH:2 * HSH], Act.Tanh)
            sig_o = sb.tile([B, HSH], f32, tag="sig_o")
            nc.scalar.activation(sig_o[:], gp[:, 2 * HSH:G3], Act.Sigmoid)
            nc.vector.tensor_mul(y_sb[:, 0:HSH], sig_i[:], tanh_g[:])
            tanh_c = sb.tile([B, HSH], f32, tag="tanh_c")
            nc.scalar.activation(tanh_c[:], y_sb[:, 0:HSH], Act.Tanh)
            nc.vector.tensor_mul(y_sb[:, HSH:2 * HSH], sig_o[:], tanh_c[:])
            nc.sync.dma_start(y[:], y_sb[:])

    nc.compile()
    return nc


def _make_in_maps(inputs, embedding, Wx, b):
    import concourse.mybir as mybir

    np_bf16 = mybir.dt.np(mybir.dt.bfloat16)

    # Per-core static blocks (wx layout + aux row), cached across calls for
    # the same Wx/b arrays.
    key = (id(Wx), id(b))
    static = _cache.get("static")
    if static is None or _cache.get("static_key") != key:
        wx_list, aux_list = [], []
        for k in range(N_CORES):
            sl = slice(k * HSH, (k + 1) * HSH)
            # gate columns of Wx for this core: i, g, o (f unused: c0 == 0)
            wx_k = np.concatenate(
                [Wx[:, 0 * H:1 * H][:, sl], Wx[:, 2 * H:3 * H][:, sl],
                 Wx[:, 3 * H:4 * H][:, sl]], axis=1)  # [E, G3]
            # [E, G3] -> [KCH, 128, G3] -> [128, KCH*G3]
            wx_k = np.ascontiguousarray(
                wx_k.reshape(KCH, 128, G3).transpose(1, 0, 2).reshape(
                    128, KCH * G3).astype(np_bf16))
            brow = np.concatenate(
                [b[0 * H:1 * H][sl], b[2 * H:3 * H][sl], b[3 * H:4 * H][sl],
                 np.ones((B,), np.float32)])
            aux_list.append(np.ascontiguousarray(
                brow.astype(np_bf16).reshape(1, G3 + B)))
            wx_list.append(wx_k)
        static = (wx_list, aux_list)
        _cache["static"] = static
        _cache["static_key"] = key
    wx_list, aux_list = static

    # First-token embedding rows, bf16, contraction-major:
    # xt[p, c*64 + i] = emb[tok_i, c*128 + p]
    x = embedding[inputs[:, 0]].astype(np_bf16)          # [B, E]
    xt = np.ascontiguousarray(
        x.T.reshape(KCH, 128, B).transpose(1, 0, 2).reshape(128, XTW))

    in_maps = []
    for k in range(N_CORES):
        blob = np.concatenate([xt, wx_list[k]], axis=1)  # [128, BLOBW]
        in_maps.append({"blob": np.ascontiguousarray(blob),
                        "aux": aux_list[k]})
    return in_maps


def _unpack_results(results):
    c = np.empty((B, H), np.float32)
    h = np.empty((B, H), np.float32)
    for k in range(N_CORES):
        sl = slice(k * HSH, (k + 1) * HSH)
        c[:, sl] = results[k]["y"][:, 0:HSH]
        h[:, sl] = results[k]["y"][:, HSH:2 * HSH]
    return c, h


def _run_t1(inputs, embedding, Wx, b):
    from concourse.bass_utils import run_bass_kernel_spmd

    if "t1" not in _cache:
        _cache["t1"] = _build_t1_program()
    nc = _cache["t1"]
    in_maps = _make_in_maps(inputs, embedding, Wx, b)
    res = run_bass_kernel_spmd(nc, in_maps, core_ids=list(range(N_CORES)))
    return _unpack_results(res.results)


def kernel(inputs, embedding, Wx, Wh, b):
    inputs = np.asarray(inputs)
    embedding = np.asarray(embedding, dtype=np.float32)
    Wx = np.asarray(Wx, dtype=np.float32)
    Wh = np.asarray(Wh, dtype=np.float32)
    b = np.asarray(b, dtype=np.float32)

    # Exact host-side computation of how many scan steps can change state:
    # sequence b freezes forever after its first step with
    # embedding[token, EOS_ID] != 0.
    eos = np.zeros((inputs.shape[0],), bool)
    T = 0
    for t in range(inputs.shape[1]):
        eos |= embedding[inputs[:, t], EOS_ID] != 0
        T = t + 1
        if eos.all():
            break

    if T == 1:
        return _run_t1(inputs, embedding, Wx, b)
    # Probability-zero fallback (an embedding value exactly 0.0 at EOS_ID).
    return _lstm_numpy(inputs, embedding, Wx, Wh, b)
